# revision 35
# baseline (speedup 1.0000x reference)
"""Trainium2 Bass kernel for nn_NodeModel (GNN message passing).

reference:
    agg = segment_sum(edge_attr, edge_index[0], num_segments=100000)   # [N, 64]
    h = concat([x, agg, u[v_indices]], axis=1)                         # [N, 256]
    out = relu(h @ W1 + b1) @ W2 + b2                                  # [N, 128]

Strategy (8 NeuronCores, SPMD, no collectives):
  - Nodes are assigned to (core, window-of-32) slots by a degree-balanced
    snake deal: nodes sorted by degree are dealt round-robin (alternating
    direction) across all 3200 (core, window) bins, so every bin holds ~1/3200
    of all edges and almost every window needs exactly 4 zero-padded 128-edge
    tiles. Output is un-permuted on host.
  - Everything streams as bf16 (~0.4% rel err, well under the 2e-2 gate);
    the output returns as bf16 and is widened on host.
  - segment_sum via PE with the ONE-HOT as the stationary operand:
    P[e, n] = (idx[e] == n) for a 32-node window is a [128, 32] stationary
    (LDWEIGHTS = 32 cols ~ 27ns, hidden under the 64-col ea stream), and
    ea tiles are the moving operand. Each window's accumulation lands in a
    32-partition PSUM strip -> tile_size (128, 32): FOUR independent PE
    column-tiles run four windows concurrently (MMs round-robin across
    windows so adjacent instructions hit different array tiles).
  - The node-major [128, 64] per-block agg is turned feature-major by a
    PE transpose of a [128, 128] pair-of-blocks tile; the two 64-row
    halves are staged into aug with partition-shifted DVE copies.
  - MLP feature-major in bf16. agg (64 rows) and u-gather (64 rows) share one
    128-partition tile so h@W1 is 2 matmuls per output half instead of 3.
    ReLU is split between ACT (mh=0) and DVE (mh=1) to balance engines.
  - DMA granularity: one ea/x/ug/out DMA per 8-block supergroup (1024 nodes).
"""

import sys

sys.path.insert(0, "/opt/trn_rl_repo")

import numpy as np
import ml_dtypes

import concourse.bass as bass
import concourse.mybir as mybir
from concourse import bacc, tile
from concourse.bass_utils import run_bass_kernel_spmd

bf16 = ml_dtypes.bfloat16

D_X, D_E, D_U = 128, 64, 64
D_HID, D_OUT = 256, 128
NB = 128   # nodes per block
SGB = 8    # blocks per DMA supergroup

FULL_CFG = dict(
    n_cores=8, n_nodes=100000, blocks=100, group=4, wsz=32
)  # 12800 node slots/core

_cache = {}


def _build_nc(Tb, blocks, npad, group, wsz=32, n_cores=8, reps=1, opts=None):
    """Build the SPMD Bass program. Tb = per-window edge tile counts
    (shared across cores; windows are `wsz` nodes, WIN windows per block).

    reps > 1 wraps the computation in a hardware For_i loop — used only
    for timing (per-iter time = delta(wall)/delta(reps), cancelling the
    host dispatch overhead)."""
    opts = dict(opts or {})
    probe = opts.get("probe")                # None | "pe" | "dma" | ...
    ea_bufs = opts.get("ea_bufs", 5)
    p_bufs = opts.get("p_bufs", 6)
    ea_rings = opts.get("ea_rings", 2)
    ea_chunks = opts.get("ea_chunks", 2)   # ea DMA chunks/sg, alternating rings
    ug_ring = opts.get("ug_ring", "scalar")
    WSZ = wsz
    WIN = NB // WSZ
    Tb = list(Tb)
    offs = [0]
    for t in Tb:
        offs.append(offs[-1] + t)
    TT = offs[-1]
    max_blk_tiles = max(
        sum(Tb[b * WIN : (b + 1) * WIN]) for b in range(blocks)
    )
    first_sg = opts.get("first_sg", blocks % SGB if blocks % SGB else SGB)
    sg_starts = [0] + list(range(first_sg, blocks, SGB))
    sgs = [
        (s, min(s + (first_sg if s == 0 else SGB), blocks))
        for s in sg_starts
    ]
    assert sgs[-1][1] == blocks
    max_sg_tiles = max(
        sum(Tb[s * WIN : e * WIN]) for s, e in sgs
    )
    nc = bacc.Bacc(
        "TRN2", target_bir_lowering=False, debug=False, num_devices=n_cores
    )
    f32, b16 = mybir.dt.float32, mybir.dt.bfloat16

    GW = group * NB       # nodes per MLP group
    SGW = SGB * NB        # nodes per supergroup
    gpsg = SGB // group   # MLP groups per supergroup

    # partition-major layouts; [K, mh, M] for weights
    ea_in = nc.declare_dram_parameter("ea", [128, TT * 64], b16, isOutput=False)
    idx_in = nc.declare_dram_parameter("idx", [128, TT], b16, isOutput=False)
    iota_in = nc.declare_dram_parameter("iota", [128, WSZ], b16, isOutput=False)
    ident_in = nc.declare_dram_parameter("ident", [128, 128], b16, isOutput=False)
    xT_in = nc.declare_dram_parameter("xT", [128, npad], b16, isOutput=False)
    ugT_in = nc.declare_dram_parameter("ugT", [64, npad], b16, isOutput=False)
    w1x_in = nc.declare_dram_parameter("w1x", [128, 2, 128], b16, isOutput=False)
    w1au_in = nc.declare_dram_parameter("w1au", [128, 2, 128], b16, isOutput=False)
    w2_in = nc.declare_dram_parameter("w2", [128, 2, 128], b16, isOutput=False)
    b1_in = nc.declare_dram_parameter("b1", [128, 2], f32, isOutput=False)
    b2_in = nc.declare_dram_parameter("b2", [128, 1], f32, isOutput=False)
    outT = nc.declare_dram_parameter("outT", [128, npad], b16, isOutput=True)

    with tile.TileContext(nc) as tc:
        sg_bufs = opts.get("sg_bufs", 5)
        with (
            tc.tile_pool(name="const", bufs=1) as cpool,
            tc.tile_pool(name="x", bufs=sg_bufs) as xpool,
            tc.tile_pool(name="ea", bufs=ea_bufs) as eapool,
            tc.tile_pool(name="p", bufs=p_bufs) as ppool,
            tc.tile_pool(name="aggn", bufs=3) as aggnpool,
            tc.tile_pool(name="aug", bufs=sg_bufs) as augpool,
            tc.tile_pool(name="h1", bufs=4) as h1pool,
            tc.tile_pool(name="outs", bufs=sg_bufs) as opool,
            tc.tile_pool(name="ps_agg", bufs=3, space="PSUM") as agg_ps_pool,
            tc.tile_pool(name="ps_tr", bufs=1, space="PSUM") as tr_ps_pool,
            tc.tile_pool(name="ps_o1", bufs=2, space="PSUM") as o1_ps_pool,
            tc.tile_pool(name="ps_o2", bufs=2, space="PSUM") as o2_ps_pool,
        ):
          def _emit_body():
              sg_tiles = {}

              def issue_sg(sg):
                  bs, be = sgs[sg]
                  nsg = (be - bs) * NB
                  s = bs * NB
                  o_sg = offs[bs * WIN]
                  Tsg = offs[be * WIN] - o_sg
                  ea_sg = eapool.tile(
                      [128, max_sg_tiles * 64], b16, tag="ea",
                      name=f"ea{sg}"
                  )
                  rings = ([nc.sync] if ea_rings == 1
                           else [nc.sync, nc.scalar])
                  cuts = [Tsg * k // ea_chunks for k in range(ea_chunks + 1)]
                  for k in range(ea_chunks):
                      lo, hi = cuts[k] * 64, cuts[k + 1] * 64
                      if hi > lo:
                          rings[k % len(rings)].dma_start(
                              ea_sg[:, lo:hi],
                              ea_in[:, o_sg * 64 + lo : o_sg * 64 + hi],
                          )
                  ring_map = {"sync": nc.sync, "scalar": nc.scalar}
                  x_t = xpool.tile([128, SGW], b16, tag="x")
                  nc.sync.dma_start(x_t[:, :nsg], xT_in[:, s : s + nsg])
                  aug = augpool.tile([128, SGW], b16, tag="aug")
                  ring_map[ug_ring].dma_start(
                      aug[64:128, :nsg], ugT_in[:, s : s + nsg]
                  )
                  out_t = opool.tile([128, SGW], b16, tag="outs")
                  sg_tiles[sg] = (ea_sg, x_t, aug, out_t)

              # first supergroup's streams lead the ring, ahead of consts
              prefetch = opts.get("prefetch", 3)
              if probe is None:
                  issue_sg(0)
                  if prefetch >= 2 and len(sgs) > 1:
                      issue_sg(1)

              # ---- constants ----
              # On the gpsimd (SWDGE) ring: their cross-iteration buffer
              # waits must not head-of-line-block the sync/scalar HWDGE
              # FIFOs that stream ea/x/ug, or DMA serializes behind the
              # previous loop iteration's compute.
              cring = {"gpsimd": nc.gpsimd, "sync": nc.sync}[
                  opts.get("const_ring", "gpsimd")
              ]
              idx_t = cpool.tile([128, TT], b16, tag="idx")
              cring.dma_start(idx_t[:], idx_in[:])
              iota_t = cpool.tile([128, WSZ], b16, tag="iota")
              cring.dma_start(iota_t[:], iota_in[:])
              ident_t = cpool.tile([128, 128], b16, tag="ident")
              cring.dma_start(ident_t[:], ident_in[:])
              w1x_t = cpool.tile([128, 2, 128], b16, tag="w1x")
              cring.dma_start(w1x_t[:], w1x_in[:])
              w1au_t = cpool.tile([128, 2, 128], b16, tag="w1au")
              cring.dma_start(w1au_t[:], w1au_in[:])
              w2_t = cpool.tile([128, 2, 128], b16, tag="w2")
              cring.dma_start(w2_t[:], w2_in[:])
              b1_t = cpool.tile([128, 2], f32, tag="b1")
              cring.dma_start(b1_t[:], b1_in[:])
              b2_t = cpool.tile([128, 1], f32, tag="b2")
              cring.dma_start(b2_t[:], b2_in[:])

              def seg_block(ea_t, o_b, b, agg_ps):
                  """One block's segment-sum: one-hot build + window MMs
                  round-robined across the 4 PE column-tiles."""
                  Tws = Tb[b * WIN : (b + 1) * WIN]
                  Tblk = sum(Tws)
                  ob_g = offs[b * WIN]         # global tile offset
                  p_t = ppool.tile(
                      [128, max_blk_tiles, WSZ], b16, tag="p"
                  )
                  nc.vector.tensor_tensor(
                      out=p_t[:, 0:Tblk, :],
                      in0=idx_t[:, ob_g : ob_g + Tblk]
                      .unsqueeze(2)
                      .broadcast_to([128, Tblk, WSZ]),
                      in1=iota_t[:, 0:WSZ]
                      .unsqueeze(1)
                      .broadcast_to([128, Tblk, WSZ]),
                      op=mybir.AluOpType.is_equal,
                  )
                  ti_base = [sum(Tws[:w]) for w in range(WIN)]
                  for r in range(max(Tws)):
                      for w in range(WIN):
                          if r >= Tws[w]:
                              continue
                          ti = ti_base[w] + r
                          nc.tensor.matmul(
                              agg_ps[w * WSZ : (w + 1) * WSZ, :],
                              p_t[:, ti, :],
                              ea_t[:, (o_b + ti) * 64
                                   : (o_b + ti + 1) * 64],
                              start=(r == 0),
                              stop=(r == Tws[w] - 1),
                              skip_group_check=True,
                              tile_position=(0, w * WSZ),
                          )

              if probe in ("pe", "seg", "mlp"):
                  # pure PE throughput: matmul schedule on const tiles
                  ea_c = eapool.tile([128, max_blk_tiles * 64], b16, tag="ea")
                  nc.sync.dma_start(
                      ea_c[:], ea_in[:, : max_blk_tiles * 64]
                  )
                  x_c = xpool.tile([128, GW], b16, tag="x")
                  nc.sync.dma_start(x_c[:], xT_in[:, :GW])
                  h_c = h1pool.tile([128, GW], b16, tag="h1")
                  nc.scalar.activation(
                      out=h_c[:], in_=x_c[:],
                      func=mybir.ActivationFunctionType.Copy,
                  )
                  aggn_c = aggnpool.tile([128, 128], b16, tag="aggn")
                  nc.vector.tensor_copy(out=aggn_c[:], in_=ident_t[:])
                  for b in range(blocks):
                      if probe in ("pe", "seg"):
                          agg_ps = agg_ps_pool.tile([128, 64], f32, tag="agg")
                          seg_block(ea_c, 0, b, agg_ps)
                      if probe == "pe" and b % 2 == 1:
                          tr = tr_ps_pool.tile([128, 128], b16, tag="tr")
                          nc.tensor.transpose(tr[:], aggn_c[:], ident_t[:])
                      if probe != "seg" and b % group == group - 1:
                          for mh in range(2):
                              o1 = o1_ps_pool.tile([128, GW], f32, tag="o1")
                              nc.tensor.matmul(
                                  o1[:], w1x_t[:, mh, :], x_c[:],
                                  start=True, stop=False,
                              )
                              nc.tensor.matmul(
                                  o1[:], w1au_t[:, mh, :], x_c[:],
                                  start=False, stop=True,
                              )
                          o2 = o2_ps_pool.tile([128, GW], f32, tag="o2")
                          for kh in range(2):
                              nc.tensor.matmul(
                                  o2[:], w2_t[:, kh, :], h_c[:],
                                  start=(kh == 0), stop=(kh == 1),
                              )
                  return
              if probe == "pedma":
                  # contention ceiling: all streams + full compute, no
                  # cross-dependencies (compute reads const tiles only)
                  ea_c = cpool.tile([128, max_blk_tiles * 64], b16, tag="eac")
                  nc.sync.dma_start(ea_c[:], ea_in[:, : max_blk_tiles * 64])
                  x_c = cpool.tile([128, GW], b16, tag="xc")
                  nc.sync.dma_start(x_c[:], xT_in[:, :GW])
                  h_c = cpool.tile([128, GW], b16, tag="hc")
                  nc.scalar.activation(
                      out=h_c[:], in_=x_c[:],
                      func=mybir.ActivationFunctionType.Copy,
                  )
                  aggn_c = cpool.tile([128, 128], b16, tag="aggnc")
                  nc.vector.tensor_copy(out=aggn_c[:], in_=ident_t[:])
                  out_c = cpool.tile([128, SGW], b16, tag="outc")
                  nc.vector.tensor_copy(out=out_c[:, 0:GW], in_=h_c[:])
                  nc.vector.tensor_copy(out=out_c[:, GW:SGW], in_=h_c[:])
                  for sg2 in range(len(sgs)):
                      issue_sg(sg2)
                  for b in range(blocks):
                      agg_ps = agg_ps_pool.tile([128, 64], f32, tag="agg")
                      seg_block(ea_c, 0, b, agg_ps)
                      if b % 2 == 1:
                          tr = tr_ps_pool.tile([128, 128], b16, tag="tr")
                          nc.tensor.transpose(tr[:], aggn_c[:], ident_t[:])
                      if b % group == group - 1:
                          for mh in range(2):
                              o1 = o1_ps_pool.tile([128, GW], f32, tag="o1")
                              nc.tensor.matmul(
                                  o1[:], w1x_t[:, mh, :], x_c[:],
                                  start=True, stop=False,
                              )
                              nc.tensor.matmul(
                                  o1[:], w1au_t[:, mh, :], x_c[:],
                                  start=False, stop=True,
                              )
                          o2 = o2_ps_pool.tile([128, GW], f32, tag="o2")
                          for kh in range(2):
                              nc.tensor.matmul(
                                  o2[:], w2_t[:, kh, :], h_c[:],
                                  start=(kh == 0), stop=(kh == 1),
                              )
                  for sg2, (bs2, be2) in enumerate(sgs):
                      nsg2 = (be2 - bs2) * NB
                      nc.scalar.dma_start(
                          outT[:, bs2 * NB : bs2 * NB + nsg2],
                          out_c[:, :nsg2],
                      )
                  return
              if probe == "dma":
                  # pure DMA floor: all input streams, no compute
                  for sg in range(len(sgs)):
                      issue_sg(sg)
                  return

              for sg, (bs, be) in enumerate(sgs):
                  nsg = (be - bs) * NB
                  s = bs * NB
                  o_sg = offs[bs * WIN]
                  # ---- supergroup DMAs (prefetched ahead of compute) ----
                  if sg not in sg_tiles:
                      issue_sg(sg)
                  if sg + prefetch < len(sgs) and (
                      sg + prefetch not in sg_tiles
                  ):
                      issue_sg(sg + prefetch)
                  ea_sg, x_t, aug, out_t = sg_tiles.pop(sg)

                  # ---- software-pipelined per-sg schedule ----
                  # segsum MM batches run back-to-back; each block-pair's
                  # transpose is emitted one extra block later (its ACT
                  # inputs are then long done), DVE copies right after it,
                  # and the MLPs come after ALL of the sg's segsum MMs so
                  # the aug tile is complete before PE reaches them.
                  nblk = be - bs
                  aggn_of = {}

                  def emit_transpose(k):
                      aggn_k = aggn_of.pop(k)
                      tr = tr_ps_pool.tile([128, 128], b16, tag="tr")
                      nc.tensor.transpose(tr[:], aggn_k[:], ident_t[:])
                      c0 = 2 * k * NB
                      # on ACT, not DVE: DVE must stay a pure p-build
                      # stream (these copies wait on PE transposes and
                      # would head-of-line-block the next p-build)
                      nc.scalar.activation(
                          out=aug[0:64, c0 : c0 + NB], in_=tr[0:64, :],
                          func=mybir.ActivationFunctionType.Copy,
                      )
                      nc.scalar.activation(
                          out=aug[0:64, c0 + NB : c0 + 2 * NB],
                          in_=tr[64:128, :],
                          func=mybir.ActivationFunctionType.Copy,
                      )

                  pending = []
                  aggn = None
                  for j in range(nblk):
                      b = bs + j
                      o_b = offs[b * WIN] - o_sg   # tile offset in ea_sg
                      agg_ps = agg_ps_pool.tile([128, 64], f32, tag="agg")
                      seg_block(ea_sg, o_b, b, agg_ps)
                      # stage node-major agg into the pair tile (bf16)
                      if j % 2 == 0:
                          aggn = aggnpool.tile([128, 128], b16, tag="aggn")
                      nc.scalar.activation(
                          out=aggn[:, (j % 2) * 64 : (j % 2) * 64 + 64],
                          in_=agg_ps[:],
                          func=mybir.ActivationFunctionType.Copy,
                      )
                      if j % 2 == 1:
                          aggn_of[j // 2] = aggn
                          pending.append(j // 2)
                      while pending and 2 * pending[0] + 2 <= j:
                          emit_transpose(pending.pop(0))
                  for k in pending:
                      emit_transpose(k)

                  # ---- MLPs for this supergroup ----
                  for g in range(gpsg):
                      gb = bs + g * group
                      if gb >= be:
                          break
                      gw = (min(gb + group, be) - gb) * NB
                      go = g * group * NB    # offset within supergroup
                      h1_list = []
                      for mh in range(2):
                          o1 = o1_ps_pool.tile([128, GW], f32, tag="o1")
                          nc.tensor.matmul(
                              o1[:, :gw], w1x_t[:, mh, :],
                              x_t[:, go : go + gw],
                              start=True, stop=False,
                          )
                          nc.tensor.matmul(
                              o1[:, :gw], w1au_t[:, mh, :],
                              aug[:, go : go + gw],
                              start=False, stop=True,
                          )
                          h1 = h1pool.tile([128, GW], b16, tag="h1")
                          # both halves on ACT: DVE stays p-build-only
                          nc.scalar.activation(
                              out=h1[:, :gw], in_=o1[:, :gw],
                              func=mybir.ActivationFunctionType.Relu,
                              bias=b1_t[:, mh : mh + 1],
                          )
                          h1_list.append(h1)
                      o2 = o2_ps_pool.tile([128, GW], f32, tag="o2")
                      for kh in range(2):
                          nc.tensor.matmul(
                              o2[:, :gw], w2_t[:, kh, :],
                              h1_list[kh][:, :gw],
                              start=(kh == 0), stop=(kh == 1),
                          )
                      nc.scalar.activation(
                          out=out_t[:, go : go + gw], in_=o2[:, :gw],
                          func=mybir.ActivationFunctionType.Identity,
                          bias=b2_t[:],
                      )
                  nc.scalar.dma_start(
                      outT[:, s : s + nsg], out_t[:, :nsg]
                  )

          if reps == 1:
              _emit_body()
          else:
              with tc.For_i(0, reps, 1):
                  _emit_body()

    nc.compile()
    return nc


def _pack_inputs(x, edge_index, edge_attr, u, v_indices, W1, b1, W2, b2, cfg):
    """Host-side sharding: degree-balanced node permutation + edge packing."""
    n_cores, blocks = cfg["n_cores"], cfg["blocks"]
    n_nodes = cfg["n_nodes"]
    WSZ = cfg.get("wsz", 32)
    WIN = NB // WSZ
    npad = blocks * NB
    nwin = npad // WSZ           # windows per core
    nbins = n_cores * nwin       # (core, window) bins
    nslots = nbins * WSZ
    row = np.asarray(edge_index[0], dtype=np.int64)
    ea = np.asarray(edge_attr, dtype=np.float32)
    x = np.asarray(x, dtype=np.float32)
    u = np.asarray(u, dtype=np.float32)
    v_indices = np.asarray(v_indices, dtype=np.int64)
    W1 = np.asarray(W1, dtype=np.float32)
    W2 = np.asarray(W2, dtype=np.float32)
    b1 = np.asarray(b1, dtype=np.float32)
    b2 = np.asarray(b2, dtype=np.float32)
    d_e = ea.shape[1]

    # ---- snake-deal nodes (sorted by degree desc) across bins ----
    deg = np.bincount(row, minlength=n_nodes)
    order = np.argsort(-deg, kind="stable")          # high degree first
    node_core = np.empty(n_nodes, np.int32)
    node_win = np.empty(n_nodes, np.int32)
    node_off = np.empty(n_nodes, np.int32)
    pos = np.arange(nslots)
    rounds, cols = pos // nbins, pos % nbins
    bins = np.where(rounds % 2 == 0, cols, nbins - 1 - cols)
    rb, bb = rounds[:n_nodes], bins[:n_nodes]
    node_core[order] = (bb // nwin).astype(np.int32)
    node_win[order] = (bb % nwin).astype(np.int32)
    node_off[order] = rb.astype(np.int32)
    node_plocal = node_win * WSZ + node_off          # slot within core

    # ---- edge buckets ----
    ec = node_core[row]
    ew = node_win[row]
    em = node_off[row]
    key = ec.astype(np.int64) * nwin + ew
    cnt = np.bincount(key, minlength=nbins).reshape(n_cores, nwin)
    Tb = np.maximum(1, (cnt.max(axis=0) + 127) // 128).astype(int)  # [nwin]
    offs = np.concatenate([[0], np.cumsum(Tb)])
    TT = int(offs[-1])

    order_e = np.argsort(key, kind="stable")
    key_s = key[order_e]
    cnt_flat = np.bincount(key_s, minlength=nbins)
    starts_flat = np.concatenate([[0], np.cumsum(cnt_flat)])[:-1]
    rank = np.arange(len(key_s)) - starts_flat[key_s]
    ew_s = ew[order_e]
    slot = offs[ew_s] * 128 + rank                   # within-core slot
    ec_s = ec[order_e]
    em_s = em[order_e].astype(np.float32)
    ea_hi = ea[order_e].astype(bf16)

    ea_pack = np.empty((n_cores, 128, TT * 64), dtype=bf16)
    idx_pack = np.empty((n_cores, 128, TT), dtype=bf16)
    for c in range(n_cores):
        m = ec_s == c
        coreslots = np.zeros((TT * 128, d_e), dtype=bf16)
        coreslots[slot[m]] = ea_hi[m]
        ea_pack[c] = (
            coreslots.reshape(TT, 128, d_e).transpose(1, 0, 2).reshape(128, -1)
        )
        ivals = np.zeros(TT * 128, dtype=np.float32)
        ivals[slot[m]] = em_s[m]
        idx_pack[c] = ivals.reshape(TT, 128).T.astype(bf16)

    iota = np.broadcast_to(
        np.arange(WSZ, dtype=np.float32), (128, WSZ)
    ).astype(bf16)
    ident = np.eye(128, dtype=np.float32).astype(bf16)
    uT = u.T  # [d_u, n_graphs]

    w1x = np.ascontiguousarray(W1[:D_X].reshape(D_X, 2, 128)).astype(bf16)
    w1au = np.ascontiguousarray(W1[D_X:].reshape(128, 2, 128)).astype(bf16)
    w2 = np.ascontiguousarray(
        W2.reshape(2, 128, D_OUT).transpose(1, 0, 2)
    ).astype(bf16)
    b1p = np.ascontiguousarray(b1.reshape(2, 128).T)
    b2p = np.ascontiguousarray(b2.reshape(128, 1))

    in_maps = []
    for c in range(n_cores):
        sel = node_core == c
        pl = node_plocal[sel]
        xT = np.zeros((D_X, npad), dtype=bf16)
        xT[:, pl] = x[sel].T.astype(bf16)
        ugT = np.zeros((D_U, npad), dtype=bf16)
        ugT[:, pl] = uT[:, v_indices[sel]].astype(bf16)
        in_maps.append({
            "ea": ea_pack[c],
            "idx": idx_pack[c],
            "iota": iota,
            "ident": ident,
            "xT": xT,
            "ugT": ugT,
            "w1x": w1x,
            "w1au": w1au,
            "w2": w2,
            "b1": b1p,
            "b2": b2p,
        })
    unperm = (node_core, node_plocal)
    return in_maps, tuple(int(t) for t in Tb), unperm


def _unpack_output(res_per_core, unperm, cfg):
    node_core, node_plocal = unperm
    n_nodes = cfg["n_nodes"]
    out = np.empty((n_nodes, D_OUT), dtype=np.float32)
    for c in range(cfg["n_cores"]):
        sel = node_core == c
        out[sel] = np.asarray(res_per_core[c]).astype(np.float32).T[
            node_plocal[sel]
        ]
    return out


def _run(inputs, cfg, reps=1):
    in_maps, T, unperm = _pack_inputs(
        inputs["x"], inputs["edge_index"], inputs["edge_attr"], inputs["u"],
        inputs["v_indices"], inputs["W1"], inputs["b1"], inputs["W2"],
        inputs["b2"], cfg,
    )
    key = (T, cfg["blocks"], cfg["group"], cfg.get("wsz", 32), reps)
    if key not in _cache:
        _cache[key] = _build_nc(
            T, cfg["blocks"], cfg["blocks"] * NB, cfg["group"],
            wsz=cfg.get("wsz", 32), n_cores=cfg["n_cores"], reps=reps,
        )
    nc = _cache[key]
    res = run_bass_kernel_spmd(nc, in_maps, list(range(cfg["n_cores"])))
    return _unpack_output(
        [res.results[c]["outT"] for c in range(cfg["n_cores"])], unperm, cfg
    )


def kernel(x, edge_index, edge_attr, u, v_indices, W1, b1, W2, b2):
    inputs = dict(x=x, edge_index=edge_index, edge_attr=edge_attr, u=u,
                  v_indices=v_indices, W1=W1, b1=b1, W2=W2, b2=b2)
    return _run(inputs, FULL_CFG)


# revision 39
# speedup vs baseline: 1.0078x; 1.0078x over previous
"""Trainium2 Bass kernel for nn_NodeModel (GNN message passing).

reference:
    agg = segment_sum(edge_attr, edge_index[0], num_segments=100000)   # [N, 64]
    h = concat([x, agg, u[v_indices]], axis=1)                         # [N, 256]
    out = relu(h @ W1 + b1) @ W2 + b2                                  # [N, 128]

Strategy (8 NeuronCores, SPMD, no collectives):
  - Nodes are assigned to (core, window-of-32) slots by a degree-balanced
    snake deal: nodes sorted by degree are dealt round-robin (alternating
    direction) across all 3200 (core, window) bins, so every bin holds ~1/3200
    of all edges and almost every window needs exactly 4 zero-padded 128-edge
    tiles. Output is un-permuted on host.
  - Everything streams as bf16 (~0.4% rel err, well under the 2e-2 gate);
    the output returns as bf16 and is widened on host.
  - segment_sum via PE with the ONE-HOT as the stationary operand:
    P[e, n] = (idx[e] == n) for a 32-node window is a [128, 32] stationary
    (LDWEIGHTS = 32 cols ~ 27ns, hidden under the 64-col ea stream), and
    ea tiles are the moving operand. Each window's accumulation lands in a
    32-partition PSUM strip -> tile_size (128, 32): FOUR independent PE
    column-tiles run four windows concurrently (MMs round-robin across
    windows so adjacent instructions hit different array tiles).
  - The node-major [128, 64] per-block agg is turned feature-major by a
    PE transpose of a [128, 128] pair-of-blocks tile; the two 64-row
    halves are staged into aug with partition-shifted DVE copies.
  - MLP feature-major in bf16. agg (64 rows) and u-gather (64 rows) share one
    128-partition tile so h@W1 is 2 matmuls per output half instead of 3.
    ReLU is split between ACT (mh=0) and DVE (mh=1) to balance engines.
  - DMA granularity: one ea/x/ug/out DMA per 8-block supergroup (1024 nodes).
"""

import sys

sys.path.insert(0, "/opt/trn_rl_repo")

import numpy as np
import ml_dtypes

import concourse.bass as bass
import concourse.mybir as mybir
from concourse import bacc, tile
from concourse.bass_utils import run_bass_kernel_spmd

bf16 = ml_dtypes.bfloat16

D_X, D_E, D_U = 128, 64, 64
D_HID, D_OUT = 256, 128
NB = 128   # nodes per block
SGB = 8    # blocks per DMA supergroup

FULL_CFG = dict(
    n_cores=8, n_nodes=100000, blocks=100, group=4, wsz=32
)  # 12800 node slots/core

_cache = {}


def _build_nc(Tb, blocks, npad, group, wsz=32, n_cores=8, reps=1, opts=None):
    """Build the SPMD Bass program. Tb = per-window edge tile counts
    (shared across cores; windows are `wsz` nodes, WIN windows per block).

    reps > 1 wraps the computation in a hardware For_i loop — used only
    for timing (per-iter time = delta(wall)/delta(reps), cancelling the
    host dispatch overhead)."""
    opts = dict(opts or {})
    probe = opts.get("probe")                # None | "pe" | "dma" | ...
    ea_bufs = opts.get("ea_bufs", 3)
    p_bufs = opts.get("p_bufs", 6)
    ea_rings = opts.get("ea_rings", 2)
    ea_chunks = opts.get("ea_chunks", 2)   # ea DMA chunks/sg, alternating rings
    ug_ring = opts.get("ug_ring", "scalar")
    WSZ = wsz
    WIN = NB // WSZ
    Tb = list(Tb)
    offs = [0]
    for t in Tb:
        offs.append(offs[-1] + t)
    TT = offs[-1]
    max_blk_tiles = max(
        sum(Tb[b * WIN : (b + 1) * WIN]) for b in range(blocks)
    )
    first_sg = opts.get("first_sg", blocks % SGB if blocks % SGB else SGB)
    sg_starts = [0] + list(range(first_sg, blocks, SGB))
    sgs = [
        (s, min(s + (first_sg if s == 0 else SGB), blocks))
        for s in sg_starts
    ]
    assert sgs[-1][1] == blocks
    max_sg_tiles = max(
        sum(Tb[s * WIN : e * WIN]) for s, e in sgs
    )
    nc = bacc.Bacc(
        "TRN2", target_bir_lowering=False, debug=False, num_devices=n_cores
    )
    f32, b16 = mybir.dt.float32, mybir.dt.bfloat16

    GW = group * NB       # nodes per MLP group
    SGW = SGB * NB        # nodes per supergroup
    gpsg = SGB // group   # MLP groups per supergroup

    # partition-major layouts; [K, mh, M] for weights
    ea_in = nc.declare_dram_parameter("ea", [128, TT * 64], b16, isOutput=False)
    idx_in = nc.declare_dram_parameter("idx", [128, TT], b16, isOutput=False)
    iota_in = nc.declare_dram_parameter("iota", [128, WSZ], b16, isOutput=False)
    ident_in = nc.declare_dram_parameter("ident", [128, 128], b16, isOutput=False)
    xT_in = nc.declare_dram_parameter("xT", [128, npad], b16, isOutput=False)
    ugT_in = nc.declare_dram_parameter("ugT", [64, npad], b16, isOutput=False)
    w1x_in = nc.declare_dram_parameter("w1x", [128, 2, 128], b16, isOutput=False)
    w1au_in = nc.declare_dram_parameter("w1au", [128, 2, 128], b16, isOutput=False)
    w2_in = nc.declare_dram_parameter("w2", [128, 2, 128], b16, isOutput=False)
    b1_in = nc.declare_dram_parameter("b1", [128, 2], f32, isOutput=False)
    b2_in = nc.declare_dram_parameter("b2", [128, 1], f32, isOutput=False)
    outT = nc.declare_dram_parameter("outT", [128, npad], b16, isOutput=True)

    with tile.TileContext(nc) as tc:
        sg_bufs = opts.get("sg_bufs", 3)
        with (
            tc.tile_pool(name="const", bufs=1) as cpool,
            tc.tile_pool(name="x", bufs=sg_bufs) as xpool,
            tc.tile_pool(name="ea", bufs=ea_bufs) as eapool,
            tc.tile_pool(name="p", bufs=p_bufs) as ppool,
            tc.tile_pool(name="aggn", bufs=3) as aggnpool,
            tc.tile_pool(name="aug", bufs=sg_bufs) as augpool,
            tc.tile_pool(name="h1", bufs=4) as h1pool,
            tc.tile_pool(name="outs", bufs=sg_bufs) as opool,
            tc.tile_pool(name="ps_agg", bufs=3, space="PSUM") as agg_ps_pool,
            tc.tile_pool(name="ps_tr", bufs=1, space="PSUM") as tr_ps_pool,
            tc.tile_pool(name="ps_o1", bufs=2, space="PSUM") as o1_ps_pool,
            tc.tile_pool(name="ps_o2", bufs=2, space="PSUM") as o2_ps_pool,
        ):
          def _emit_body():
              sg_tiles = {}

              def issue_sg(sg):
                  bs, be = sgs[sg]
                  nsg = (be - bs) * NB
                  s = bs * NB
                  o_sg = offs[bs * WIN]
                  Tsg = offs[be * WIN] - o_sg
                  ea_sg = eapool.tile(
                      [128, max_sg_tiles * 64], b16, tag="ea",
                      name=f"ea{sg}"
                  )
                  rings = ([nc.sync] if ea_rings == 1
                           else [nc.sync, nc.scalar])
                  cuts = [Tsg * k // ea_chunks for k in range(ea_chunks + 1)]
                  for k in range(ea_chunks):
                      lo, hi = cuts[k] * 64, cuts[k + 1] * 64
                      if hi > lo:
                          rings[k % len(rings)].dma_start(
                              ea_sg[:, lo:hi],
                              ea_in[:, o_sg * 64 + lo : o_sg * 64 + hi],
                          )
                  ring_map = {"sync": nc.sync, "scalar": nc.scalar}
                  x_t = xpool.tile([128, SGW], b16, tag="x")
                  nc.sync.dma_start(x_t[:, :nsg], xT_in[:, s : s + nsg])
                  aug = augpool.tile([128, SGW], b16, tag="aug")
                  ring_map[ug_ring].dma_start(
                      aug[64:128, :nsg], ugT_in[:, s : s + nsg]
                  )
                  out_t = opool.tile([128, SGW], b16, tag="outs")
                  sg_tiles[sg] = (ea_sg, x_t, aug, out_t)

              # first supergroup's streams lead the ring, ahead of consts
              prefetch = opts.get("prefetch", 2)
              if probe is None:
                  issue_sg(0)
                  if prefetch >= 2 and len(sgs) > 1:
                      issue_sg(1)

              # ---- constants ----
              # On the gpsimd (SWDGE) ring: their cross-iteration buffer
              # waits must not head-of-line-block the sync/scalar HWDGE
              # FIFOs that stream ea/x/ug, or DMA serializes behind the
              # previous loop iteration's compute.
              cring = {"gpsimd": nc.gpsimd, "sync": nc.sync}[
                  opts.get("const_ring", "gpsimd")
              ]
              idx_t = cpool.tile([128, TT], b16, tag="idx")
              cring.dma_start(idx_t[:], idx_in[:])
              iota_t = cpool.tile([128, WSZ], b16, tag="iota")
              cring.dma_start(iota_t[:], iota_in[:])
              ident_t = cpool.tile([128, 128], b16, tag="ident")
              cring.dma_start(ident_t[:], ident_in[:])
              w1x_t = cpool.tile([128, 2, 128], b16, tag="w1x")
              cring.dma_start(w1x_t[:], w1x_in[:])
              w1au_t = cpool.tile([128, 2, 128], b16, tag="w1au")
              cring.dma_start(w1au_t[:], w1au_in[:])
              w2_t = cpool.tile([128, 2, 128], b16, tag="w2")
              cring.dma_start(w2_t[:], w2_in[:])
              b1_t = cpool.tile([128, 2], f32, tag="b1")
              cring.dma_start(b1_t[:], b1_in[:])
              b2_t = cpool.tile([128, 1], f32, tag="b2")
              cring.dma_start(b2_t[:], b2_in[:])

              def seg_block(ea_t, o_b, b, agg_ps):
                  """One block's segment-sum: one-hot build + window MMs
                  round-robined across the 4 PE column-tiles."""
                  Tws = Tb[b * WIN : (b + 1) * WIN]
                  Tblk = sum(Tws)
                  ob_g = offs[b * WIN]         # global tile offset
                  p_t = ppool.tile(
                      [128, max_blk_tiles, WSZ], b16, tag="p"
                  )
                  nc.vector.tensor_tensor(
                      out=p_t[:, 0:Tblk, :],
                      in0=idx_t[:, ob_g : ob_g + Tblk]
                      .unsqueeze(2)
                      .broadcast_to([128, Tblk, WSZ]),
                      in1=iota_t[:, 0:WSZ]
                      .unsqueeze(1)
                      .broadcast_to([128, Tblk, WSZ]),
                      op=mybir.AluOpType.is_equal,
                  )
                  ti_base = [sum(Tws[:w]) for w in range(WIN)]
                  for r in range(max(Tws)):
                      for w in range(WIN):
                          if r >= Tws[w]:
                              continue
                          ti = ti_base[w] + r
                          nc.tensor.matmul(
                              agg_ps[w * WSZ : (w + 1) * WSZ, :],
                              p_t[:, ti, :],
                              ea_t[:, (o_b + ti) * 64
                                   : (o_b + ti + 1) * 64],
                              start=(r == 0),
                              stop=(r == Tws[w] - 1),
                              skip_group_check=True,
                              tile_position=(0, w * WSZ),
                          )

              if probe in ("pe", "seg", "mlp"):
                  # pure PE throughput: matmul schedule on const tiles
                  ea_c = eapool.tile([128, max_blk_tiles * 64], b16, tag="ea")
                  nc.sync.dma_start(
                      ea_c[:], ea_in[:, : max_blk_tiles * 64]
                  )
                  x_c = xpool.tile([128, GW], b16, tag="x")
                  nc.sync.dma_start(x_c[:], xT_in[:, :GW])
                  h_c = h1pool.tile([128, GW], b16, tag="h1")
                  nc.scalar.activation(
                      out=h_c[:], in_=x_c[:],
                      func=mybir.ActivationFunctionType.Copy,
                  )
                  aggn_c = aggnpool.tile([128, 128], b16, tag="aggn")
                  nc.vector.tensor_copy(out=aggn_c[:], in_=ident_t[:])
                  for b in range(blocks):
                      if probe in ("pe", "seg"):
                          agg_ps = agg_ps_pool.tile([128, 64], f32, tag="agg")
                          seg_block(ea_c, 0, b, agg_ps)
                      if probe == "pe" and b % 2 == 1:
                          tr = tr_ps_pool.tile([128, 128], b16, tag="tr")
                          nc.tensor.transpose(tr[:], aggn_c[:], ident_t[:])
                      if probe != "seg" and b % group == group - 1:
                          for mh in range(2):
                              o1 = o1_ps_pool.tile([128, GW], f32, tag="o1")
                              nc.tensor.matmul(
                                  o1[:], w1x_t[:, mh, :], x_c[:],
                                  start=True, stop=False,
                              )
                              nc.tensor.matmul(
                                  o1[:], w1au_t[:, mh, :], x_c[:],
                                  start=False, stop=True,
                              )
                          o2 = o2_ps_pool.tile([128, GW], f32, tag="o2")
                          for kh in range(2):
                              nc.tensor.matmul(
                                  o2[:], w2_t[:, kh, :], h_c[:],
                                  start=(kh == 0), stop=(kh == 1),
                              )
                  return
              if probe == "pedma":
                  # contention ceiling: all streams + full compute, no
                  # cross-dependencies (compute reads const tiles only)
                  ea_c = cpool.tile([128, max_blk_tiles * 64], b16, tag="eac")
                  nc.sync.dma_start(ea_c[:], ea_in[:, : max_blk_tiles * 64])
                  x_c = cpool.tile([128, GW], b16, tag="xc")
                  nc.sync.dma_start(x_c[:], xT_in[:, :GW])
                  h_c = cpool.tile([128, GW], b16, tag="hc")
                  nc.scalar.activation(
                      out=h_c[:], in_=x_c[:],
                      func=mybir.ActivationFunctionType.Copy,
                  )
                  aggn_c = cpool.tile([128, 128], b16, tag="aggnc")
                  nc.vector.tensor_copy(out=aggn_c[:], in_=ident_t[:])
                  out_c = cpool.tile([128, SGW], b16, tag="outc")
                  nc.vector.tensor_copy(out=out_c[:, 0:GW], in_=h_c[:])
                  nc.vector.tensor_copy(out=out_c[:, GW:SGW], in_=h_c[:])
                  for sg2 in range(len(sgs)):
                      issue_sg(sg2)
                  for b in range(blocks):
                      agg_ps = agg_ps_pool.tile([128, 64], f32, tag="agg")
                      seg_block(ea_c, 0, b, agg_ps)
                      if b % 2 == 1:
                          tr = tr_ps_pool.tile([128, 128], b16, tag="tr")
                          nc.tensor.transpose(tr[:], aggn_c[:], ident_t[:])
                      if b % group == group - 1:
                          for mh in range(2):
                              o1 = o1_ps_pool.tile([128, GW], f32, tag="o1")
                              nc.tensor.matmul(
                                  o1[:], w1x_t[:, mh, :], x_c[:],
                                  start=True, stop=False,
                              )
                              nc.tensor.matmul(
                                  o1[:], w1au_t[:, mh, :], x_c[:],
                                  start=False, stop=True,
                              )
                          o2 = o2_ps_pool.tile([128, GW], f32, tag="o2")
                          for kh in range(2):
                              nc.tensor.matmul(
                                  o2[:], w2_t[:, kh, :], h_c[:],
                                  start=(kh == 0), stop=(kh == 1),
                              )
                  for sg2, (bs2, be2) in enumerate(sgs):
                      nsg2 = (be2 - bs2) * NB
                      nc.scalar.dma_start(
                          outT[:, bs2 * NB : bs2 * NB + nsg2],
                          out_c[:, :nsg2],
                      )
                  return
              if probe == "dma":
                  # pure DMA floor: all input streams, no compute
                  for sg in range(len(sgs)):
                      issue_sg(sg)
                  return

              for sg, (bs, be) in enumerate(sgs):
                  nsg = (be - bs) * NB
                  s = bs * NB
                  o_sg = offs[bs * WIN]
                  # ---- supergroup DMAs (prefetched ahead of compute) ----
                  if sg not in sg_tiles:
                      issue_sg(sg)
                  if sg + prefetch < len(sgs) and (
                      sg + prefetch not in sg_tiles
                  ):
                      issue_sg(sg + prefetch)
                  ea_sg, x_t, aug, out_t = sg_tiles.pop(sg)

                  # ---- software-pipelined per-sg schedule ----
                  # segsum MM batches run back-to-back; each block-pair's
                  # transpose is emitted one extra block later (its ACT
                  # inputs are then long done), DVE copies right after it,
                  # and the MLPs come after ALL of the sg's segsum MMs so
                  # the aug tile is complete before PE reaches them.
                  nblk = be - bs
                  aggn_of = {}

                  def emit_transpose(k):
                      aggn_k = aggn_of.pop(k)
                      tr = tr_ps_pool.tile([128, 128], b16, tag="tr")
                      nc.tensor.transpose(tr[:], aggn_k[:], ident_t[:])
                      c0 = 2 * k * NB
                      # on ACT, not DVE: DVE must stay a pure p-build
                      # stream (these copies wait on PE transposes and
                      # would head-of-line-block the next p-build)
                      nc.scalar.activation(
                          out=aug[0:64, c0 : c0 + NB], in_=tr[0:64, :],
                          func=mybir.ActivationFunctionType.Copy,
                      )
                      nc.scalar.activation(
                          out=aug[0:64, c0 + NB : c0 + 2 * NB],
                          in_=tr[64:128, :],
                          func=mybir.ActivationFunctionType.Copy,
                      )

                  pending = []
                  aggn = None
                  for j in range(nblk):
                      b = bs + j
                      o_b = offs[b * WIN] - o_sg   # tile offset in ea_sg
                      agg_ps = agg_ps_pool.tile([128, 64], f32, tag="agg")
                      seg_block(ea_sg, o_b, b, agg_ps)
                      # stage node-major agg into the pair tile (bf16)
                      if j % 2 == 0:
                          aggn = aggnpool.tile([128, 128], b16, tag="aggn")
                      nc.scalar.activation(
                          out=aggn[:, (j % 2) * 64 : (j % 2) * 64 + 64],
                          in_=agg_ps[:],
                          func=mybir.ActivationFunctionType.Copy,
                      )
                      if j % 2 == 1:
                          aggn_of[j // 2] = aggn
                          pending.append(j // 2)
                      while pending and 2 * pending[0] + 2 <= j:
                          emit_transpose(pending.pop(0))
                  for k in pending:
                      emit_transpose(k)

                  # ---- MLPs for this supergroup ----
                  for g in range(gpsg):
                      gb = bs + g * group
                      if gb >= be:
                          break
                      gw = (min(gb + group, be) - gb) * NB
                      go = g * group * NB    # offset within supergroup
                      h1_list = []
                      for mh in range(2):
                          o1 = o1_ps_pool.tile([128, GW], f32, tag="o1")
                          nc.tensor.matmul(
                              o1[:, :gw], w1x_t[:, mh, :],
                              x_t[:, go : go + gw],
                              start=True, stop=False,
                          )
                          nc.tensor.matmul(
                              o1[:, :gw], w1au_t[:, mh, :],
                              aug[:, go : go + gw],
                              start=False, stop=True,
                          )
                          h1 = h1pool.tile([128, GW], b16, tag="h1")
                          # mh0 on ACT; mh1 on DVE — sheds ~0.5us/sg from
                          # the loaded ACT queue, and at the sg TAIL the
                          # DVE FIFO block of next-sg p-builds is covered
                          # by the o2/out matmuls still ahead on PE
                          if mh == 0:
                              nc.scalar.activation(
                                  out=h1[:, :gw], in_=o1[:, :gw],
                                  func=mybir.ActivationFunctionType.Relu,
                                  bias=b1_t[:, mh : mh + 1],
                              )
                          else:
                              nc.vector.tensor_scalar(
                                  out=h1[:, :gw], in0=o1[:, :gw],
                                  scalar1=b1_t[:, 1:2], scalar2=0.0,
                                  op0=mybir.AluOpType.add,
                                  op1=mybir.AluOpType.max,
                              )
                          h1_list.append(h1)
                      o2 = o2_ps_pool.tile([128, GW], f32, tag="o2")
                      for kh in range(2):
                          nc.tensor.matmul(
                              o2[:, :gw], w2_t[:, kh, :],
                              h1_list[kh][:, :gw],
                              start=(kh == 0), stop=(kh == 1),
                          )
                      nc.scalar.activation(
                          out=out_t[:, go : go + gw], in_=o2[:, :gw],
                          func=mybir.ActivationFunctionType.Identity,
                          bias=b2_t[:],
                      )
                  nc.scalar.dma_start(
                      outT[:, s : s + nsg], out_t[:, :nsg]
                  )

          if reps == 1:
              _emit_body()
          else:
              with tc.For_i(0, reps, 1):
                  _emit_body()

    nc.compile()
    return nc


def _pack_inputs(x, edge_index, edge_attr, u, v_indices, W1, b1, W2, b2, cfg):
    """Host-side sharding: degree-balanced node permutation + edge packing."""
    n_cores, blocks = cfg["n_cores"], cfg["blocks"]
    n_nodes = cfg["n_nodes"]
    WSZ = cfg.get("wsz", 32)
    WIN = NB // WSZ
    npad = blocks * NB
    nwin = npad // WSZ           # windows per core
    nbins = n_cores * nwin       # (core, window) bins
    nslots = nbins * WSZ
    row = np.asarray(edge_index[0], dtype=np.int64)
    ea = np.asarray(edge_attr, dtype=np.float32)
    x = np.asarray(x, dtype=np.float32)
    u = np.asarray(u, dtype=np.float32)
    v_indices = np.asarray(v_indices, dtype=np.int64)
    W1 = np.asarray(W1, dtype=np.float32)
    W2 = np.asarray(W2, dtype=np.float32)
    b1 = np.asarray(b1, dtype=np.float32)
    b2 = np.asarray(b2, dtype=np.float32)
    d_e = ea.shape[1]

    # ---- snake-deal nodes (sorted by degree desc) across bins ----
    deg = np.bincount(row, minlength=n_nodes)
    order = np.argsort(-deg, kind="stable")          # high degree first
    node_core = np.empty(n_nodes, np.int32)
    node_win = np.empty(n_nodes, np.int32)
    node_off = np.empty(n_nodes, np.int32)
    pos = np.arange(nslots)
    rounds, cols = pos // nbins, pos % nbins
    bins = np.where(rounds % 2 == 0, cols, nbins - 1 - cols)
    rb, bb = rounds[:n_nodes], bins[:n_nodes]
    node_core[order] = (bb // nwin).astype(np.int32)
    node_win[order] = (bb % nwin).astype(np.int32)
    node_off[order] = rb.astype(np.int32)
    node_plocal = node_win * WSZ + node_off          # slot within core

    # ---- edge buckets ----
    ec = node_core[row]
    ew = node_win[row]
    em = node_off[row]
    key = ec.astype(np.int64) * nwin + ew
    cnt = np.bincount(key, minlength=nbins).reshape(n_cores, nwin)
    Tb = np.maximum(1, (cnt.max(axis=0) + 127) // 128).astype(int)  # [nwin]
    offs = np.concatenate([[0], np.cumsum(Tb)])
    TT = int(offs[-1])

    order_e = np.argsort(key, kind="stable")
    key_s = key[order_e]
    cnt_flat = np.bincount(key_s, minlength=nbins)
    starts_flat = np.concatenate([[0], np.cumsum(cnt_flat)])[:-1]
    rank = np.arange(len(key_s)) - starts_flat[key_s]
    ew_s = ew[order_e]
    slot = offs[ew_s] * 128 + rank                   # within-core slot
    ec_s = ec[order_e]
    em_s = em[order_e].astype(np.float32)
    ea_hi = ea[order_e].astype(bf16)

    ea_pack = np.empty((n_cores, 128, TT * 64), dtype=bf16)
    idx_pack = np.empty((n_cores, 128, TT), dtype=bf16)
    for c in range(n_cores):
        m = ec_s == c
        coreslots = np.zeros((TT * 128, d_e), dtype=bf16)
        coreslots[slot[m]] = ea_hi[m]
        ea_pack[c] = (
            coreslots.reshape(TT, 128, d_e).transpose(1, 0, 2).reshape(128, -1)
        )
        ivals = np.zeros(TT * 128, dtype=np.float32)
        ivals[slot[m]] = em_s[m]
        idx_pack[c] = ivals.reshape(TT, 128).T.astype(bf16)

    iota = np.broadcast_to(
        np.arange(WSZ, dtype=np.float32), (128, WSZ)
    ).astype(bf16)
    ident = np.eye(128, dtype=np.float32).astype(bf16)
    uT = u.T  # [d_u, n_graphs]

    w1x = np.ascontiguousarray(W1[:D_X].reshape(D_X, 2, 128)).astype(bf16)
    w1au = np.ascontiguousarray(W1[D_X:].reshape(128, 2, 128)).astype(bf16)
    w2 = np.ascontiguousarray(
        W2.reshape(2, 128, D_OUT).transpose(1, 0, 2)
    ).astype(bf16)
    b1p = np.ascontiguousarray(b1.reshape(2, 128).T)
    b2p = np.ascontiguousarray(b2.reshape(128, 1))

    in_maps = []
    for c in range(n_cores):
        sel = node_core == c
        pl = node_plocal[sel]
        xT = np.zeros((D_X, npad), dtype=bf16)
        xT[:, pl] = x[sel].T.astype(bf16)
        ugT = np.zeros((D_U, npad), dtype=bf16)
        ugT[:, pl] = uT[:, v_indices[sel]].astype(bf16)
        in_maps.append({
            "ea": ea_pack[c],
            "idx": idx_pack[c],
            "iota": iota,
            "ident": ident,
            "xT": xT,
            "ugT": ugT,
            "w1x": w1x,
            "w1au": w1au,
            "w2": w2,
            "b1": b1p,
            "b2": b2p,
        })
    unperm = (node_core, node_plocal)
    return in_maps, tuple(int(t) for t in Tb), unperm


def _unpack_output(res_per_core, unperm, cfg):
    node_core, node_plocal = unperm
    n_nodes = cfg["n_nodes"]
    out = np.empty((n_nodes, D_OUT), dtype=np.float32)
    for c in range(cfg["n_cores"]):
        sel = node_core == c
        out[sel] = np.asarray(res_per_core[c]).astype(np.float32).T[
            node_plocal[sel]
        ]
    return out


def _run(inputs, cfg, reps=1):
    in_maps, T, unperm = _pack_inputs(
        inputs["x"], inputs["edge_index"], inputs["edge_attr"], inputs["u"],
        inputs["v_indices"], inputs["W1"], inputs["b1"], inputs["W2"],
        inputs["b2"], cfg,
    )
    key = (T, cfg["blocks"], cfg["group"], cfg.get("wsz", 32), reps)
    if key not in _cache:
        _cache[key] = _build_nc(
            T, cfg["blocks"], cfg["blocks"] * NB, cfg["group"],
            wsz=cfg.get("wsz", 32), n_cores=cfg["n_cores"], reps=reps,
        )
    nc = _cache[key]
    res = run_bass_kernel_spmd(nc, in_maps, list(range(cfg["n_cores"])))
    return _unpack_output(
        [res.results[c]["outT"] for c in range(cfg["n_cores"])], unperm, cfg
    )


def kernel(x, edge_index, edge_attr, u, v_indices, W1, b1, W2, b2):
    inputs = dict(x=x, edge_index=edge_index, edge_attr=edge_attr, u=u,
                  v_indices=v_indices, W1=W1, b1=b1, W2=W2, b2=b2)
    return _run(inputs, FULL_CFG)
